# revision 19
# baseline (speedup 1.0000x reference)
"""Trainium2 Bass kernel for nn_Decoder (GRU + decoder heads).

Math per time step t (per batch element):
    gi = x_t @ W_ih.T + b_ih          # [3H]  (r,z,n)
    gh = h   @ W_hh.T + b_hh          # [3H]
    r = sigmoid(gi_r + gh_r); z = sigmoid(gi_z + gh_z)
    n = tanh(gi_n + r * gh_n)
    h' = (1-z)*n + z*h
    phi  = relu(h' @ W_phi.T + b_phi)
    mean = sigmoid(phi @ W_mean.T + b_mean)
    std  = softplus(phi @ W_std.T + b_std)
    xs   = eps_t * std + mean

Sharding: data-parallel over batch, 1024 = 8 cores x 128.

Device layout is fully "transposed": features on SBUF partitions, batch on
the free dimension.  All matmuls keep weights stationary (lhsT) and stream
batch columns.  Biases are folded into PSUM accumulation via an augmented
ones-row in the x tile (K=33 / K=1 matmuls), so sigmoid/tanh need no bias.
softplus lives in a different ACT table set than sigmoid/tanh, so the std
head is buffered (pre-activation) in SBUF and finished in a post-phase with
a single table switch; xs is computed there too.
"""

import numpy as np

import concourse.bass as bass
import concourse.mybir as mybir
from concourse.tile import TileContext
from concourse.bass_utils import run_bass_kernel_spmd

AF = mybir.ActivationFunctionType
OP = mybir.AluOpType
BF16 = mybir.dt.bfloat16
F32 = mybir.dt.float32
NP_BF16 = mybir.dt.np(BF16)

B_TOT, T_FULL, Z, H, D, X = 1024, 256, 32, 256, 256, 64
NCORES = 8
B = B_TOT // NCORES          # 128 batch per core
HC = H // 128                # 2 feature chunks of the hidden state
SB = 4                       # head block (phi/ms) size in steps
SX = 8                       # x-input DMA block size in steps
PC = 32                      # post-phase chunk size in steps


def split_sync_waits(nc, limit: int = 1):
    """The walrus build in this container allows only `limit` sync-wait
    commands per instruction; move excess waits onto preceding
    EventSemaphore ops on the same engine (engine streams are FIFO, so the
    semantics are identical)."""
    for f in nc.m.functions:
        for blk in f.blocks:
            new = []
            for inst in blk.instructions:
                si = inst.sync_info
                if si is not None and si.on_wait and len(si.on_wait) > limit:
                    waits = list(si.on_wait)
                    keep, extra = waits[-limit:], waits[:-limit]
                    for j, w in enumerate(extra):
                        ev = mybir.InstEventSemaphore(
                            name=f"{inst.name}-prw{j}", ins=[], outs=[])
                        ev.engine = inst.engine
                        ev.sync_info = mybir.SyncInfo(on_wait=[w], on_update=[])
                        nc.register_instruction(ev)
                        new.append(ev)
                    si.on_wait = keep
                new.append(inst)
            blk.instructions[:] = new


def build_nc(T: int = T_FULL, do_head: bool = True, do_post: bool = True):
    """Build the per-core Bass program (identical on all 8 cores)."""
    nc = bass.Bass()

    # ---- DRAM I/O ----
    x_d = nc.dram_tensor("x_t", [Z, T, B], BF16, kind="ExternalInput")
    eps_d = nc.dram_tensor("eps_t", [X, T, B], BF16, kind="ExternalInput")
    # All bf16 weights packed in one tensor (single DMA keeps the wait count
    # on the first consuming matmul under the hardware limit):
    #   cols 0:1024    w_gib: rows 0:32 = W_ih.T (cols 0:768), row 32 = biases
    #                  (0:512 b_ih+b_hh r,z | 512:768 b_ih_n | 768:1024 row32
    #                   = b_hh_n, rows 0:32 zero)
    #   cols 1024:2560 w_hh:  k*768+m*128+j = W_hh.T[k*128+p, m*128+j]
    #   cols 2560:3072 w_phi: k*256+f = W_phi.T[k*128+p, f]
    #   cols 3072:3328 w_ms:  k*128+f = W_ms.T[k*128+p, f],
    #                  W_ms = concat([W_std, W_mean]) (std rows 0:64)
    wall_d = nc.dram_tensor("w_all", [128, 3584], BF16, kind="ExternalInput")
    # biases for the heads (fp32): col0/1 = b_phi chunk0/1,
    # col2 = b_std replicated to both partition halves, col3 = b_mean repl.
    bias_d = nc.dram_tensor("b_pack", [128, 5], F32, kind="ExternalInput")

    xs_d = nc.dram_tensor("xs_o", [X, T, B], BF16, kind="ExternalOutput")
    mean_d = nc.dram_tensor("mean_o", [X, T, B], BF16, kind="ExternalOutput")
    std_d = nc.dram_tensor("std_o", [X, T, B], BF16, kind="ExternalOutput")

    SXc = min(SX, T)
    NB = T // SB                 # head blocks
    NB2 = (NB + 1) // 2          # packed column-block pairs
    PB = NB2 * SB * B            # packed post-buffer width

    with TileContext(nc) as tc:
        with (
            tc.tile_pool(name="const", bufs=1) as cpool,
            tc.tile_pool(name="xin", bufs=2) as xpool,
            tc.tile_pool(name="hist", bufs=2) as hpool,
            tc.tile_pool(name="gate", bufs=2) as gpool,
            tc.tile_pool(name="head", bufs=2) as dpool,
            tc.tile_pool(name="big", bufs=1) as bigpool,
            tc.tile_pool(name="post", bufs=2) as ppool,
            tc.tile_pool(name="psR", bufs=2, space="PSUM") as psR,
            tc.tile_pool(name="psZ", bufs=2, space="PSUM") as psZ,
            tc.tile_pool(name="psB", bufs=2, space="PSUM") as psB,
            tc.tile_pool(name="psH", bufs=2, space="PSUM") as psH,
        ):
            # ---- constants into SBUF ----
            wall = cpool.tile([128, 3584], BF16, name="wall")
            nc.sync.dma_start(wall, wall_d[:, :])
            wgib = wall[0:33, 0:1024]
            whh = wall[:, 1024:2560]
            wphi = wall[:, 2560:3072]
            wms = wall[:, 3072:3328]
            bphi_row = wall[0:1, 3328:3584]  # b_phi as K=1 stationary rows
            bpk = cpool.tile([128, 5], F32, name="bpk")
            nc.sync.dma_start(bpk, bias_d[:, :])

            # std pre-act / mean, packed two head-blocks per 512-col group
            # (even block -> partitions 0:64, odd -> 64:128) so the post
            # phase runs full-width [128, .] activations.
            buf_std = bigpool.tile([128, PB], BF16, name="buf_std")
            buf_mean = bigpool.tile([128, PB], BF16, name="buf_mean")
            # eps prefetched into the same packed layout (input: overlaps
            # the whole recurrence on the gpsimd DMA queue).
            eps_sb = bigpool.tile([128, PB], BF16, name="eps_sb")
            eps_v = eps_d.rearrange("x (p two s) b -> x p two (s b)",
                                    two=2, s=SB) if NB >= 2 else None
            if eps_v is not None:
                nc.gpsimd.dma_start(eps_sb[0:64, 0 : (NB // 2) * SB * B],
                                    eps_v[:, :, 0, :])
                nc.gpsimd.dma_start(eps_sb[64:128, 0 : (NB // 2) * SB * B],
                                    eps_v[:, :, 1, :])
            if NB % 2 == 1:  # odd tail block -> even half of last group
                nc.gpsimd.dma_start(
                    eps_sb[0:64, (NB2 - 1) * SB * B : NB2 * SB * B],
                    eps_d[:, (NB - 1) * SB : NB * SB, :])

            ones4 = cpool.tile([1, SB * B], BF16, name="ones4")
            nc.gpsimd.memset(ones4[:, :], 1.0)

            # h(-1) = 0
            hist_prev = hpool.tile([128, HC, SB * B], BF16, tag="hist", name="hist_i")
            nc.gpsimd.memset(hist_prev[:, :, :], 0.0)

            x_blk = None
            n_sb = None
            # deferred head state: block b's heads are issued during steps of
            # block b+1 so they fill tensor-engine idle instead of blocking the
            # recurrent chain.
            pend_phi = None   # (p_phi tiles, hist tile, block idx)
            pend_ms = None    # (phi_sb, block idx)

            def emit_phi(hist_t, b_i):
                """phi matmuls for a finished block (PE) — issued after the
                current step's h-tiles so they run during elementwise."""
                p_phi = [psH.tile([128, SB * B], F32, tag="phi", name="p_phi")
                         for _ in range(2)]
                for m in range(2):
                    for k in range(HC):
                        nc.tensor.matmul(
                            p_phi[m],
                            wphi[:, k * 256 + m * 128 : k * 256 + (m + 1) * 128],
                            hist_t[:, k, :], start=(k == 0), stop=False)
                for m in range(2):  # + b_phi via K=1 ones-row matmul
                    nc.tensor.matmul(p_phi[m],
                                     bphi_row[:, m * 128 : (m + 1) * 128],
                                     ones4, start=False, stop=True)
                phi_sb = dpool.tile([128, 2, SB * B], BF16, tag="phi_sb",
                                    name="phi_sb")
                for m in range(2):  # relu evac on DVE (ACT stays gate-only)
                    nc.vector.tensor_scalar_max(phi_sb[:, m, :], p_phi[m], 0.0)
                return phi_sb

            def emit_ms(phi_sb, b_i):
                # shares the psH pool: ms allocation cycles onto a phi buffer
                # freed once the relu evac has consumed it
                p_ms = psH.tile([128, SB * B], F32, tag="phi", name="p_ms")
                for k in range(2):
                    nc.tensor.matmul(p_ms, wms[:, k * 128 : (k + 1) * 128],
                                     phi_sb[:, k, :], start=(k == 0),
                                     stop=(k == 1))
                pr = (b_i % 2) * 64
                cols = slice((b_i // 2) * SB * B, (b_i // 2 + 1) * SB * B)
                # std pre-act: just add bias, softplus later (packed layout)
                nc.vector.tensor_scalar_add(buf_std[pr : pr + 64, cols],
                                            p_ms[0:64, :],
                                            bpk[pr : pr + 64, 2:3])
                # mean: final value
                nc.scalar.activation(buf_mean[pr : pr + 64, cols],
                                     p_ms[64:128, :], AF.Sigmoid,
                                     bias=bpk[pr : pr + 64, 3:4])
                nc.sync.dma_start(mean_d[:, b_i * SB : (b_i + 1) * SB, :],
                                  buf_mean[pr : pr + 64, cols])

            for t in range(T):
                b = t // SB          # head block index
                s = t % SB           # slot within head block
                if t % SXc == 0:
                    x_blk = xpool.tile([33, SXc * B], BF16, tag="x", name="x_blk")
                    nc.sync.dma_start(x_blk[0:32, :], x_d[:, t : t + SXc, :])
                    nc.gpsimd.memset(x_blk[32:33, :], 1.0)
                if s == 0:
                    hist = hpool.tile([128, HC, SB * B], BF16, tag="hist", name="hist")
                xa = x_blk[:, (t % SXc) * B : (t % SXc + 1) * B]      # [33, B]
                x1 = x_blk[32:33, (t % SXc) * B : (t % SXc + 1) * B]  # [1, B] ones
                sp = (t - 1) % SB
                hsrc = hist_prev if s == 0 else hist
                h_prev = [hsrc[:, k, sp * B : (sp + 1) * B] for k in range(HC)]
                h_prev_m = hsrc[:, :, sp * B : (sp + 1) * B]   # [128, 2, B]

                # PSUM: p_r, p_z separate tiles so sigmoid(r) only waits on
                # r-writers (tile-granularity dependency tracking).
                p_r = psR.tile([128, 2 * B], F32, tag="r", name="p_r")
                p_z = psZ.tile([128, 2 * B], F32, tag="z", name="p_z")
                p_nh = psB.tile([128, 4 * B], F32, tag="nh", name="p_nh")

                # ---- x-side matmuls first: no h dependency, so they execute
                # during the previous step's elementwise window ----
                for m in range(2):   # r chunks
                    nc.tensor.matmul(p_r[:, m * B : (m + 1) * B],
                                     wgib[:, m * 128 : (m + 1) * 128], xa,
                                     start=(m == 0), stop=False)
                for m in range(2):   # z chunks
                    nc.tensor.matmul(p_z[:, m * B : (m + 1) * B],
                                     wgib[:, (2 + m) * 128 : (3 + m) * 128], xa,
                                     start=(m == 0), stop=False)
                for c in range(HC):  # i_n (incl b_in via ones row)
                    nc.tensor.matmul(
                        p_nh[:, c * B : (c + 1) * B],
                        wgib[:, 512 + c * 128 : 512 + (c + 1) * 128], xa,
                        start=(c == 0), stop=False)
                for c in range(HC):  # b_hn via K=1 ones-row matmul
                    nc.tensor.matmul(
                        p_nh[:, (HC + c) * B : (HC + c + 1) * B],
                        wgib[32:33, 768 + c * 128 : 768 + (c + 1) * 128], x1,
                        start=False, stop=False)

                # ---- h-side tiles, ordered by downstream chain depth:
                # r first (feeds sigmoid->mul->add->tanh->...), then h_n,
                # then z (shallow consumers) ----
                for m in range(2):   # r
                    for k in range(HC):
                        nc.tensor.matmul(
                            p_r[:, m * B : (m + 1) * B],
                            whh[:, k * 768 + m * 128 : k * 768 + (m + 1) * 128],
                            h_prev[k], start=False,
                            stop=(m == 1 and k == HC - 1))
                for c in range(HC):  # h_n
                    for k in range(HC):
                        nc.tensor.matmul(
                            p_nh[:, (HC + c) * B : (HC + c + 1) * B],
                            whh[:, k * 768 + 512 + c * 128 : k * 768 + 512 + (c + 1) * 128],
                            h_prev[k], start=False,
                            stop=(c == HC - 1 and k == HC - 1))
                for m in range(2):   # z
                    for k in range(HC):
                        nc.tensor.matmul(
                            p_z[:, m * B : (m + 1) * B],
                            whh[:, k * 768 + (2 + m) * 128 : k * 768 + (3 + m) * 128],
                            h_prev[k], start=False,
                            stop=(m == 1 and k == HC - 1))

                # ---- gate elementwise (bf16, chain-ordered) ----
                # ACT: sigmoid_r -> sigmoid_z -> tanh; DVE: rh, s, zh, e, h'
                r_sb = gpool.tile([128, 2 * B], BF16, tag="r_sb", name="r_sb")
                nc.scalar.activation(r_sb, p_r, AF.Sigmoid)
                z_sb = gpool.tile([128, 2 * B], BF16, tag="z_sb", name="z_sb")
                nc.scalar.activation(z_sb, p_z, AF.Sigmoid)
                rh = gpool.tile([128, 2 * B], BF16, tag="rh", name="rh")
                nc.vector.tensor_mul(rh, r_sb, p_nh[:, 2 * B : 4 * B])
                s_sb = gpool.tile([128, 2 * B], BF16, tag="s_sb", name="s_sb")
                nc.vector.tensor_add(s_sb, rh, p_nh[:, 0 : 2 * B])
                # zh = z*h and zm1 = z-1 on gpsimd: off the DVE queue so the
                # critical rh->s chain is never reordered behind them, and
                # they run while tanh executes.
                zh = gpool.tile([128, 2 * B], BF16, tag="zh", name="zh")
                nc.gpsimd.tensor_mul(zh, z_sb, h_prev_m)
                # zm1 = z-1 on the scalar engine (gpsimd tensor_scalar is
                # ~3.8us on hw; ACT identity slots into its idle window)
                zm1 = gpool.tile([128, 2 * B], BF16, tag="zm1", name="zm1")
                nc.scalar.activation(zm1, z_sb, AF.Identity, bias=bpk[:, 4:5])
                n_sb = gpool.tile([128, 2 * B], BF16, tag="n_sb", name="n_sb")
                nc.scalar.activation(n_sb, s_sb, AF.Tanh)
                # h' = zh - (z-1)*n
                e_sb = gpool.tile([128, 2 * B], BF16, tag="e_sb", name="e_sb")
                nc.vector.tensor_mul(e_sb, zm1, n_sb)
                h_new = hist[:, :, s * B : (s + 1) * B]
                nc.vector.tensor_sub(h_new, zh, e_sb)

                # ---- deferred heads fill the tensor idle window; emitted
                # after the gate chain so sigmoid/tanh stay ahead of the
                # head ACTs in the scalar-engine FIFO ----
                if s == 0 and pend_phi is not None and do_head:
                    hist_t, b_i = pend_phi
                    pend_ms = (emit_phi(hist_t, b_i), b_i)
                    pend_phi = None
                elif s == 1 and pend_ms is not None and do_head:
                    phi_sb_t, b_i = pend_ms
                    emit_ms(phi_sb_t, b_i)
                    pend_ms = None

                if s == SB - 1:
                    pend_phi = (hist, b)
                    hist_prev = hist

            # drain the deferred head pipeline for the last block(s)
            if do_head:
                if pend_phi is not None:
                    hist_t, b_i = pend_phi
                    pend_ms = (emit_phi(hist_t, b_i), b_i)
                    pend_phi = None
                if pend_ms is not None:
                    phi_sb_t, b_i = pend_ms
                    emit_ms(phi_sb_t, b_i)
                    pend_ms = None

            # ---- post-phase: softplus(std), xs = eps*std + mean, all in the
            # packed [128, .] layout (two head-blocks per 512-col group).
            # exp/ln share natural_log_exp_and_others: one table switch in,
            # one back out.
            std_v = std_d.rearrange("x (p two s) b -> x p two (s b)",
                                    two=2, s=SB) if NB >= 2 else None
            xs_v = xs_d.rearrange("x (p two s) b -> x p two (s b)",
                                  two=2, s=SB) if NB >= 2 else None
            GB = SB * B  # 512-col packed group
            pcc = min(8 * GB, PB)
            for c0 in (range(0, PB, pcc) if do_post else []):
                c1 = min(c0 + pcc, PB)
                cols = slice(c0, c1)
                n_el = c1 - c0
                ex = ppool.tile([128, n_el], BF16, tag="ex", name="ex")
                nc.scalar.activation(ex, buf_std[:, cols], AF.Exp)
                stg = ppool.tile([128, n_el], BF16, tag="stg", name="stg")
                nc.scalar.activation(stg, ex, AF.Ln, bias=1.0)
                xs_sb = ppool.tile([128, n_el], BF16, tag="xs", name="xs_sb")
                nc.vector.tensor_mul(xs_sb, eps_sb[:, cols], stg)
                nc.vector.tensor_add(xs_sb, xs_sb, buf_mean[:, cols])
                j0, j1 = c0 // GB, c1 // GB
                if std_v is not None and j1 * 2 <= NB:
                    nc.sync.dma_start(std_v[:, j0:j1, 0, :], stg[0:64, :])
                    nc.sync.dma_start(std_v[:, j0:j1, 1, :], stg[64:128, :])
                    nc.sync.dma_start(xs_v[:, j0:j1, 0, :], xs_sb[0:64, :])
                    nc.sync.dma_start(xs_v[:, j0:j1, 1, :], xs_sb[64:128, :])
                else:
                    # tail with an odd final block: per-group DMAs
                    for j in range(j0, j1):
                        rel = slice((j - j0) * GB, (j - j0 + 1) * GB)
                        for half in range(2):
                            bb = 2 * j + half
                            if bb >= NB:
                                break
                            pr = half * 64
                            nc.sync.dma_start(
                                std_d[:, bb * SB : (bb + 1) * SB, :],
                                stg[pr : pr + 64, rel])
                            nc.sync.dma_start(
                                xs_d[:, bb * SB : (bb + 1) * SB, :],
                                xs_sb[pr : pr + 64, rel])

    split_sync_waits(nc)
    return nc


def prep_weights(W_ih, W_hh, b_ih, b_hh, W_phi, b_phi, W_mean, b_mean, W_std,
                 b_std):
    """Host-side packing of weights into device layouts (all bf16/fp32)."""
    w_gib = np.zeros((33, 1024), np.float32)
    w_gib[0:32, 0:768] = W_ih.T
    w_gib[32, 0:512] = b_ih[0:512] + b_hh[0:512]
    w_gib[32, 512:768] = b_ih[512:768]
    w_gib[32, 768:1024] = b_hh[512:768]

    whhT = W_hh.T  # [H, 3H] = [256, 768]
    w_hh = np.concatenate([whhT[0:128], whhT[128:256]], axis=1)  # [128, 1536]

    wphiT = W_phi.T  # [256, 256]
    w_phi = np.concatenate([wphiT[0:128], wphiT[128:256]], axis=1)  # [128, 512]

    W_ms = np.concatenate([W_std, W_mean], axis=0)  # [128, 256], std first
    wmsT = W_ms.T  # [256, 128]
    w_ms = np.concatenate([wmsT[0:128], wmsT[128:256]], axis=1)  # [128, 256]

    b_pack = np.zeros((128, 5), np.float32)
    b_pack[:, 0] = b_phi[0:128]
    b_pack[:, 1] = b_phi[128:256]
    b_pack[0:64, 2] = b_std
    b_pack[64:128, 2] = b_std
    b_pack[0:64, 3] = b_mean
    b_pack[64:128, 3] = b_mean
    b_pack[:, 4] = -1.0

    w_all = np.zeros((128, 3584), np.float32)
    w_all[0:33, 0:1024] = w_gib
    w_all[:, 1024:2560] = w_hh
    w_all[:, 2560:3072] = w_phi
    w_all[:, 3072:3328] = w_ms
    w_all[0, 3328:3456] = b_phi[0:128]
    w_all[0, 3456:3584] = b_phi[128:256]
    return {"w_all": w_all.astype(NP_BF16), "b_pack": b_pack}


_NC_CACHE = {}


def run(inputs, T: int = T_FULL, trace: bool = False):
    """Run the kernel on 8 cores. Returns (results, BassKernelResults)."""
    if T not in _NC_CACHE:
        _NC_CACHE[T] = build_nc(T)
    nc = _NC_CACHE[T]

    wmaps = prep_weights(
        inputs["W_ih"], inputs["W_hh"], inputs["b_ih"], inputs["b_hh"],
        inputs["W_phi"], inputs["b_phi"], inputs["W_mean"], inputs["b_mean"],
        inputs["W_std"], inputs["b_std"])

    inp = np.asarray(inputs["inp"], np.float32)[:, :T, :]
    eps = np.asarray(inputs["eps"], np.float32)[:, :T, :]
    in_maps = []
    for c in range(NCORES):
        sl = slice(c * B, (c + 1) * B)
        in_maps.append({
            **wmaps,
            # [B, T, F] -> [F, T, B]
            "x_t": np.ascontiguousarray(inp[sl].transpose(2, 1, 0)).astype(NP_BF16),
            "eps_t": np.ascontiguousarray(eps[sl].transpose(2, 1, 0)).astype(NP_BF16),
        })

    res = run_bass_kernel_spmd(nc, in_maps, core_ids=list(range(NCORES)),
                               trace=trace)

    outs = []
    for name in ("xs_o", "mean_o", "std_o"):
        parts = [
            res.results[c][name].astype(np.float32).transpose(2, 1, 0)
            for c in range(NCORES)
        ]
        outs.append(np.concatenate(parts, axis=0))  # [B_TOT, T, X]
    return tuple(outs), res


def kernel(**inputs):
    outs, _ = run(inputs)
    return outs



# revision 20
# speedup vs baseline: 1.0846x; 1.0846x over previous
"""Trainium2 Bass kernel for nn_Decoder (GRU + decoder heads).

Math per time step t (per batch element):
    gi = x_t @ W_ih.T + b_ih          # [3H]  (r,z,n)
    gh = h   @ W_hh.T + b_hh          # [3H]
    r = sigmoid(gi_r + gh_r); z = sigmoid(gi_z + gh_z)
    n = tanh(gi_n + r * gh_n)
    h' = (1-z)*n + z*h
    phi  = relu(h' @ W_phi.T + b_phi)
    mean = sigmoid(phi @ W_mean.T + b_mean)
    std  = softplus(phi @ W_std.T + b_std)
    xs   = eps_t * std + mean

Sharding: data-parallel over batch, 1024 = 8 cores x 128.

Device layout is fully "transposed": features on SBUF partitions, batch on
the free dimension.  All matmuls keep weights stationary (lhsT) and stream
batch columns.  Biases are folded into PSUM accumulation via an augmented
ones-row in the x tile (K=33 / K=1 matmuls), so sigmoid/tanh need no bias.
softplus lives in a different ACT table set than sigmoid/tanh, so the std
head is buffered (pre-activation) in SBUF and finished in a post-phase with
a single table switch; xs is computed there too.
"""

import numpy as np

import concourse.bass as bass
import concourse.mybir as mybir
from concourse.tile import TileContext
from concourse.bass_utils import run_bass_kernel_spmd

AF = mybir.ActivationFunctionType
OP = mybir.AluOpType
BF16 = mybir.dt.bfloat16
F32 = mybir.dt.float32
NP_BF16 = mybir.dt.np(BF16)

B_TOT, T_FULL, Z, H, D, X = 1024, 256, 32, 256, 256, 64
NCORES = 8
B = B_TOT // NCORES          # 128 batch per core
HC = H // 128                # 2 feature chunks of the hidden state
SB = 4                       # head block (phi/ms) size in steps
SX = 8                       # x-input DMA block size in steps
PC = 32                      # post-phase chunk size in steps


def split_sync_waits(nc, limit: int = 1):
    """The walrus build in this container allows only `limit` sync-wait
    commands per instruction; move excess waits onto preceding
    EventSemaphore ops on the same engine (engine streams are FIFO, so the
    semantics are identical)."""
    for f in nc.m.functions:
        for blk in f.blocks:
            new = []
            for inst in blk.instructions:
                si = inst.sync_info
                if si is not None and si.on_wait and len(si.on_wait) > limit:
                    waits = list(si.on_wait)
                    keep, extra = waits[-limit:], waits[:-limit]
                    for j, w in enumerate(extra):
                        ev = mybir.InstEventSemaphore(
                            name=f"{inst.name}-prw{j}", ins=[], outs=[])
                        ev.engine = inst.engine
                        ev.sync_info = mybir.SyncInfo(on_wait=[w], on_update=[])
                        nc.register_instruction(ev)
                        new.append(ev)
                    si.on_wait = keep
                new.append(inst)
            blk.instructions[:] = new


def build_nc(T: int = T_FULL, do_head: bool = True, do_post: bool = True):
    """Build the per-core Bass program (identical on all 8 cores)."""
    nc = bass.Bass()

    # ---- DRAM I/O ----
    x_d = nc.dram_tensor("x_t", [Z, T, B], BF16, kind="ExternalInput")
    eps_d = nc.dram_tensor("eps_t", [X, T, B], BF16, kind="ExternalInput")
    # All bf16 weights packed in one tensor (single DMA keeps the wait count
    # on the first consuming matmul under the hardware limit):
    #   cols 0:1024    w_gib: rows 0:32 = W_ih.T (cols 0:768), row 32 = biases
    #                  (0:512 b_ih+b_hh r,z | 512:768 b_ih_n | 768:1024 row32
    #                   = b_hh_n, rows 0:32 zero)
    #   cols 1024:2560 w_hh:  k*768+m*128+j = W_hh.T[k*128+p, m*128+j]
    #   cols 2560:3072 w_phi: k*256+f = W_phi.T[k*128+p, f]
    #   cols 3072:3328 w_ms:  k*128+f = W_ms.T[k*128+p, f],
    #                  W_ms = concat([W_std, W_mean]) (std rows 0:64)
    wall_d = nc.dram_tensor("w_all", [128, 3328], BF16, kind="ExternalInput")
    # biases for the heads (fp32): col0/1 = b_phi chunk0/1,
    # col2 = b_std replicated to both partition halves, col3 = b_mean repl.
    bias_d = nc.dram_tensor("b_pack", [128, 5], F32, kind="ExternalInput")

    xs_d = nc.dram_tensor("xs_o", [X, T, B], BF16, kind="ExternalOutput")
    mean_d = nc.dram_tensor("mean_o", [X, T, B], BF16, kind="ExternalOutput")
    std_d = nc.dram_tensor("std_o", [X, T, B], BF16, kind="ExternalOutput")

    SXc = min(SX, T)
    NB = T // SB                 # head blocks
    NB2 = (NB + 1) // 2          # packed column-block pairs
    PB = NB2 * SB * B            # packed post-buffer width

    with TileContext(nc) as tc:
        with (
            tc.tile_pool(name="const", bufs=1) as cpool,
            tc.tile_pool(name="xin", bufs=2) as xpool,
            tc.tile_pool(name="hist", bufs=2) as hpool,
            tc.tile_pool(name="gate", bufs=2) as gpool,
            tc.tile_pool(name="head", bufs=2) as dpool,
            tc.tile_pool(name="big", bufs=1) as bigpool,
            tc.tile_pool(name="post", bufs=2) as ppool,
            tc.tile_pool(name="psR", bufs=2, space="PSUM") as psR,
            tc.tile_pool(name="psZ", bufs=2, space="PSUM") as psZ,
            tc.tile_pool(name="psB", bufs=2, space="PSUM") as psB,
            tc.tile_pool(name="psH", bufs=2, space="PSUM") as psH,
        ):
            # ---- constants into SBUF ----
            wall = cpool.tile([128, 3328], BF16, name="wall")
            nc.sync.dma_start(wall, wall_d[:, :])
            wgib = wall[0:33, 0:1024]
            whh = wall[:, 1024:2560]
            wphi = wall[:, 2560:3072]
            wms = wall[:, 3072:3328]
            bpk = cpool.tile([128, 5], F32, name="bpk")
            nc.sync.dma_start(bpk, bias_d[:, :])

            # std pre-act / mean, packed two head-blocks per 512-col group
            # (even block -> partitions 0:64, odd -> 64:128) so the post
            # phase runs full-width [128, .] activations.
            buf_std = bigpool.tile([128, PB], BF16, name="buf_std")
            buf_mean = bigpool.tile([128, PB], BF16, name="buf_mean")
            # eps prefetched into the same packed layout (input: overlaps
            # the whole recurrence on the gpsimd DMA queue).
            eps_sb = bigpool.tile([128, PB], BF16, name="eps_sb")
            eps_v = eps_d.rearrange("x (p two s) b -> x p two (s b)",
                                    two=2, s=SB) if NB >= 2 else None
            if eps_v is not None:
                nc.gpsimd.dma_start(eps_sb[0:64, 0 : (NB // 2) * SB * B],
                                    eps_v[:, :, 0, :])
                nc.gpsimd.dma_start(eps_sb[64:128, 0 : (NB // 2) * SB * B],
                                    eps_v[:, :, 1, :])
            if NB % 2 == 1:  # odd tail block -> even half of last group
                nc.gpsimd.dma_start(
                    eps_sb[0:64, (NB2 - 1) * SB * B : NB2 * SB * B],
                    eps_d[:, (NB - 1) * SB : NB * SB, :])

            # h(-1) = 0
            hist_prev = hpool.tile([128, HC, SB * B], BF16, tag="hist", name="hist_i")
            nc.gpsimd.memset(hist_prev[:, :, :], 0.0)

            x_blk = None
            n_sb = None
            # deferred head state: block b's heads are issued during steps of
            # block b+1 so they fill tensor-engine idle instead of blocking the
            # recurrent chain.
            pend_phi = None   # (hist tile, block idx) awaiting head pipeline
            hd_phi = None     # (p_phi tiles, phi_sb) for the block in flight
            hd_ms = None      # p_ms tile for the block in flight

            def emit_phi_mm(hist_t):
                """phi matmuls for a finished block (PE) — issued after the
                current step's h-tiles so they run during elementwise."""
                p_phi = [psH.tile([128, SB * B], F32, tag="phi", name="p_phi")
                         for _ in range(2)]
                for m in range(2):
                    for k in range(HC):
                        nc.tensor.matmul(
                            p_phi[m],
                            wphi[:, k * 256 + m * 128 : k * 256 + (m + 1) * 128],
                            hist_t[:, k, :], start=(k == 0), stop=(k == HC - 1))
                phi_sb = dpool.tile([128, 2, SB * B], BF16, tag="phi_sb",
                                    name="phi_sb")
                return p_phi, phi_sb

            def emit_relu(p_phi, phi_sb, m):
                nc.scalar.activation(phi_sb[:, m, :], p_phi[m], AF.Relu,
                                     bias=bpk[:, m : m + 1])

            def emit_ms_mm(phi_sb, b_i):
                # shares the psH pool: ms allocation cycles onto a phi buffer
                # freed once the relu evac has consumed it
                p_ms = psH.tile([128, SB * B], F32, tag="phi", name="p_ms")
                for k in range(2):
                    nc.tensor.matmul(p_ms, wms[:, k * 128 : (k + 1) * 128],
                                     phi_sb[:, k, :], start=(k == 0),
                                     stop=(k == 1))
                pr = (b_i % 2) * 64
                cols = slice((b_i // 2) * SB * B, (b_i // 2 + 1) * SB * B)
                # mean: final value
                nc.scalar.activation(buf_mean[pr : pr + 64, cols],
                                     p_ms[64:128, :], AF.Sigmoid,
                                     bias=bpk[pr : pr + 64, 3:4])
                nc.sync.dma_start(mean_d[:, b_i * SB : (b_i + 1) * SB, :],
                                  buf_mean[pr : pr + 64, cols])
                return p_ms

            def emit_std(p_ms, b_i):
                pr = (b_i % 2) * 64
                cols = slice((b_i // 2) * SB * B, (b_i // 2 + 1) * SB * B)
                # std pre-act: just add bias, softplus later (packed layout)
                nc.scalar.activation(buf_std[pr : pr + 64, cols], p_ms[0:64, :],
                                     AF.Identity, bias=bpk[pr : pr + 64, 2:3])

            for t in range(T):
                b = t // SB          # head block index
                s = t % SB           # slot within head block
                if t % SXc == 0:
                    x_blk = xpool.tile([33, SXc * B], BF16, tag="x", name="x_blk")
                    nc.sync.dma_start(x_blk[0:32, :], x_d[:, t : t + SXc, :])
                    nc.gpsimd.memset(x_blk[32:33, :], 1.0)
                if s == 0:
                    hist = hpool.tile([128, HC, SB * B], BF16, tag="hist", name="hist")
                xa = x_blk[:, (t % SXc) * B : (t % SXc + 1) * B]      # [33, B]
                x1 = x_blk[32:33, (t % SXc) * B : (t % SXc + 1) * B]  # [1, B] ones
                sp = (t - 1) % SB
                hsrc = hist_prev if s == 0 else hist
                h_prev = [hsrc[:, k, sp * B : (sp + 1) * B] for k in range(HC)]
                h_prev_m = hsrc[:, :, sp * B : (sp + 1) * B]   # [128, 2, B]

                # PSUM: p_r, p_z separate tiles so sigmoid(r) only waits on
                # r-writers (tile-granularity dependency tracking).
                p_r = psR.tile([128, 2 * B], F32, tag="r", name="p_r")
                p_z = psZ.tile([128, 2 * B], F32, tag="z", name="p_z")
                p_nh = psB.tile([128, 4 * B], F32, tag="nh", name="p_nh")

                # ---- x-side matmuls first: no h dependency, so they execute
                # during the previous step's elementwise window ----
                for m in range(2):   # r chunks
                    nc.tensor.matmul(p_r[:, m * B : (m + 1) * B],
                                     wgib[:, m * 128 : (m + 1) * 128], xa,
                                     start=(m == 0), stop=False)
                for m in range(2):   # z chunks
                    nc.tensor.matmul(p_z[:, m * B : (m + 1) * B],
                                     wgib[:, (2 + m) * 128 : (3 + m) * 128], xa,
                                     start=(m == 0), stop=False)
                for c in range(HC):  # i_n (incl b_in via ones row)
                    nc.tensor.matmul(
                        p_nh[:, c * B : (c + 1) * B],
                        wgib[:, 512 + c * 128 : 512 + (c + 1) * 128], xa,
                        start=(c == 0), stop=False)
                for c in range(HC):  # b_hn via K=1 ones-row matmul
                    nc.tensor.matmul(
                        p_nh[:, (HC + c) * B : (HC + c + 1) * B],
                        wgib[32:33, 768 + c * 128 : 768 + (c + 1) * 128], x1,
                        start=False, stop=False)

                # ---- h-side tiles, ordered by downstream chain depth:
                # r first (feeds sigmoid->mul->add->tanh->...), then h_n,
                # then z (shallow consumers) ----
                for m in range(2):   # r
                    for k in range(HC):
                        nc.tensor.matmul(
                            p_r[:, m * B : (m + 1) * B],
                            whh[:, k * 768 + m * 128 : k * 768 + (m + 1) * 128],
                            h_prev[k], start=False,
                            stop=(m == 1 and k == HC - 1))
                for c in range(HC):  # h_n
                    for k in range(HC):
                        nc.tensor.matmul(
                            p_nh[:, (HC + c) * B : (HC + c + 1) * B],
                            whh[:, k * 768 + 512 + c * 128 : k * 768 + 512 + (c + 1) * 128],
                            h_prev[k], start=False,
                            stop=(c == HC - 1 and k == HC - 1))
                for m in range(2):   # z
                    for k in range(HC):
                        nc.tensor.matmul(
                            p_z[:, m * B : (m + 1) * B],
                            whh[:, k * 768 + (2 + m) * 128 : k * 768 + (3 + m) * 128],
                            h_prev[k], start=False,
                            stop=(m == 1 and k == HC - 1))

                # ---- gate elementwise (bf16, chain-ordered) ----
                # ACT: sigmoid_r -> sigmoid_z -> tanh; DVE: rh, s, zh, e, h'
                r_sb = gpool.tile([128, 2 * B], BF16, tag="r_sb", name="r_sb")
                nc.scalar.activation(r_sb, p_r, AF.Sigmoid)
                z_sb = gpool.tile([128, 2 * B], BF16, tag="z_sb", name="z_sb")
                nc.scalar.activation(z_sb, p_z, AF.Sigmoid)
                rh = gpool.tile([128, 2 * B], BF16, tag="rh", name="rh")
                nc.vector.tensor_mul(rh, r_sb, p_nh[:, 2 * B : 4 * B])
                s_sb = gpool.tile([128, 2 * B], BF16, tag="s_sb", name="s_sb")
                nc.vector.tensor_add(s_sb, rh, p_nh[:, 0 : 2 * B])
                # zh = z*h and zm1 = z-1 on gpsimd: off the DVE queue so the
                # critical rh->s chain is never reordered behind them, and
                # they run while tanh executes.
                zh = gpool.tile([128, 2 * B], BF16, tag="zh", name="zh")
                nc.gpsimd.tensor_mul(zh, z_sb, h_prev_m)
                # zm1 = z-1 on the scalar engine (gpsimd tensor_scalar is
                # ~3.8us on hw; ACT identity slots into its idle window)
                zm1 = gpool.tile([128, 2 * B], BF16, tag="zm1", name="zm1")
                nc.scalar.activation(zm1, z_sb, AF.Identity, bias=bpk[:, 4:5])
                n_sb = gpool.tile([128, 2 * B], BF16, tag="n_sb", name="n_sb")
                nc.scalar.activation(n_sb, s_sb, AF.Tanh)
                # h' = zh - (z-1)*n
                e_sb = gpool.tile([128, 2 * B], BF16, tag="e_sb", name="e_sb")
                nc.vector.tensor_mul(e_sb, zm1, n_sb)
                h_new = hist[:, :, s * B : (s + 1) * B]
                nc.vector.tensor_sub(h_new, zh, e_sb)

                # ---- deferred heads fill idle windows; one scalar-engine
                # head op per step so tanh/sigmoid never queue behind them ----
                if do_head and pend_phi is not None:
                    hist_t, b_i = pend_phi
                    if s == 0:
                        hd_phi = emit_phi_mm(hist_t)
                        emit_relu(hd_phi[0], hd_phi[1], 0)
                    elif s == 1:
                        emit_relu(hd_phi[0], hd_phi[1], 1)
                    elif s == 2:
                        hd_ms = emit_ms_mm(hd_phi[1], b_i)
                    else:
                        emit_std(hd_ms, b_i)
                        pend_phi = None

                if s == SB - 1:
                    pend_phi = (hist, b)
                    hist_prev = hist

            # drain the deferred head pipeline for the last block
            if do_head and pend_phi is not None:
                hist_t, b_i = pend_phi
                hd_phi = emit_phi_mm(hist_t)
                emit_relu(hd_phi[0], hd_phi[1], 0)
                emit_relu(hd_phi[0], hd_phi[1], 1)
                hd_ms = emit_ms_mm(hd_phi[1], b_i)
                emit_std(hd_ms, b_i)

            # ---- post-phase: softplus(std), xs = eps*std + mean, all in the
            # packed [128, .] layout (two head-blocks per 512-col group).
            # exp/ln share natural_log_exp_and_others: one table switch in,
            # one back out.
            std_v = std_d.rearrange("x (p two s) b -> x p two (s b)",
                                    two=2, s=SB) if NB >= 2 else None
            xs_v = xs_d.rearrange("x (p two s) b -> x p two (s b)",
                                  two=2, s=SB) if NB >= 2 else None
            GB = SB * B  # 512-col packed group
            pcc = min(8 * GB, PB)
            for c0 in (range(0, PB, pcc) if do_post else []):
                c1 = min(c0 + pcc, PB)
                cols = slice(c0, c1)
                n_el = c1 - c0
                ex = ppool.tile([128, n_el], BF16, tag="ex", name="ex")
                nc.scalar.activation(ex, buf_std[:, cols], AF.Exp)
                stg = ppool.tile([128, n_el], BF16, tag="stg", name="stg")
                nc.scalar.activation(stg, ex, AF.Ln, bias=1.0)
                xs_sb = ppool.tile([128, n_el], BF16, tag="xs", name="xs_sb")
                nc.vector.tensor_mul(xs_sb, eps_sb[:, cols], stg)
                nc.vector.tensor_add(xs_sb, xs_sb, buf_mean[:, cols])
                j0, j1 = c0 // GB, c1 // GB
                if std_v is not None and j1 * 2 <= NB:
                    nc.sync.dma_start(std_v[:, j0:j1, 0, :], stg[0:64, :])
                    nc.sync.dma_start(std_v[:, j0:j1, 1, :], stg[64:128, :])
                    nc.sync.dma_start(xs_v[:, j0:j1, 0, :], xs_sb[0:64, :])
                    nc.sync.dma_start(xs_v[:, j0:j1, 1, :], xs_sb[64:128, :])
                else:
                    # tail with an odd final block: per-group DMAs
                    for j in range(j0, j1):
                        rel = slice((j - j0) * GB, (j - j0 + 1) * GB)
                        for half in range(2):
                            bb = 2 * j + half
                            if bb >= NB:
                                break
                            pr = half * 64
                            nc.sync.dma_start(
                                std_d[:, bb * SB : (bb + 1) * SB, :],
                                stg[pr : pr + 64, rel])
                            nc.sync.dma_start(
                                xs_d[:, bb * SB : (bb + 1) * SB, :],
                                xs_sb[pr : pr + 64, rel])

    split_sync_waits(nc)
    return nc


def prep_weights(W_ih, W_hh, b_ih, b_hh, W_phi, b_phi, W_mean, b_mean, W_std,
                 b_std):
    """Host-side packing of weights into device layouts (all bf16/fp32)."""
    w_gib = np.zeros((33, 1024), np.float32)
    w_gib[0:32, 0:768] = W_ih.T
    w_gib[32, 0:512] = b_ih[0:512] + b_hh[0:512]
    w_gib[32, 512:768] = b_ih[512:768]
    w_gib[32, 768:1024] = b_hh[512:768]

    whhT = W_hh.T  # [H, 3H] = [256, 768]
    w_hh = np.concatenate([whhT[0:128], whhT[128:256]], axis=1)  # [128, 1536]

    wphiT = W_phi.T  # [256, 256]
    w_phi = np.concatenate([wphiT[0:128], wphiT[128:256]], axis=1)  # [128, 512]

    W_ms = np.concatenate([W_std, W_mean], axis=0)  # [128, 256], std first
    wmsT = W_ms.T  # [256, 128]
    w_ms = np.concatenate([wmsT[0:128], wmsT[128:256]], axis=1)  # [128, 256]

    b_pack = np.zeros((128, 5), np.float32)
    b_pack[:, 0] = b_phi[0:128]
    b_pack[:, 1] = b_phi[128:256]
    b_pack[0:64, 2] = b_std
    b_pack[64:128, 2] = b_std
    b_pack[0:64, 3] = b_mean
    b_pack[64:128, 3] = b_mean
    b_pack[:, 4] = -1.0

    w_all = np.zeros((128, 3328), np.float32)
    w_all[0:33, 0:1024] = w_gib
    w_all[:, 1024:2560] = w_hh
    w_all[:, 2560:3072] = w_phi
    w_all[:, 3072:3328] = w_ms
    return {"w_all": w_all.astype(NP_BF16), "b_pack": b_pack}


_NC_CACHE = {}


def run(inputs, T: int = T_FULL, trace: bool = False):
    """Run the kernel on 8 cores. Returns (results, BassKernelResults)."""
    if T not in _NC_CACHE:
        _NC_CACHE[T] = build_nc(T)
    nc = _NC_CACHE[T]

    wmaps = prep_weights(
        inputs["W_ih"], inputs["W_hh"], inputs["b_ih"], inputs["b_hh"],
        inputs["W_phi"], inputs["b_phi"], inputs["W_mean"], inputs["b_mean"],
        inputs["W_std"], inputs["b_std"])

    inp = np.asarray(inputs["inp"], np.float32)[:, :T, :]
    eps = np.asarray(inputs["eps"], np.float32)[:, :T, :]
    in_maps = []
    for c in range(NCORES):
        sl = slice(c * B, (c + 1) * B)
        in_maps.append({
            **wmaps,
            # [B, T, F] -> [F, T, B]
            "x_t": np.ascontiguousarray(inp[sl].transpose(2, 1, 0)).astype(NP_BF16),
            "eps_t": np.ascontiguousarray(eps[sl].transpose(2, 1, 0)).astype(NP_BF16),
        })

    res = run_bass_kernel_spmd(nc, in_maps, core_ids=list(range(NCORES)),
                               trace=trace)

    outs = []
    for name in ("xs_o", "mean_o", "std_o"):
        parts = [
            res.results[c][name].astype(np.float32).transpose(2, 1, 0)
            for c in range(NCORES)
        ]
        outs.append(np.concatenate(parts, axis=0))  # [B_TOT, T, X]
    return tuple(outs), res


def kernel(**inputs):
    outs, _ = run(inputs)
    return outs

